# revision 1
# baseline (speedup 1.0000x reference)
"""Trainium2 Bass kernel for nn_LucaGPLMMultiheadAttention.

MHA with RoPE: S=2048, B=2, E=1024, H=16, hd=64, fp32.
Sharding: head-parallel across 8 cores (2 heads x 2 batch = 4 (b,h) pairs
per core). q/k/v projections column-split, out projection row-split with an
on-device ReduceScatter over the sequence axis; host concatenates shards.

All big matmuls run as float32r (fp32 streamed at full rate when the moving
free dim >= 256; TF32-like rounding, ~3e-4 rel err per matmul). The walrus
verifier requires fp32r operands to be produced by a rounding instruction,
so every matmul operand lives in an f32r-typed tile written by a DVE/ACT op.

Attention runs in transposed layouts so no probs/attn transposes are needed:
  qT/kT/vT [d, s] from projections against on-chip-transposed query
  scoresT [kj, qi] (contraction over d), exp (softmax max-sub safely skipped:
    scores are unit-scale), attnT [d+1, qi] accumulated over kj with a ones
    column in the stationary v operand yielding softmax row sums for free.
  Normalization: reciprocal row sums are partition-broadcast (idle GPSIMD)
    and multiplied into attnT per qi-block, so the out-projection is a single
    k=128 contraction and its P tiles stream out while attention continues.
bo/8 is added to every core's partial P so the ReduceScatter sum carries the
output bias and the shard DMAs DRAM->DRAM with no final SBUF pass.
"""

import os
import sys

sys.path.insert(0, "/opt/trn_rl_repo")

import numpy as np

S = 2048
B = 2
E = 1024
H = 16
HD = 64
NCORES = 8
HPC = H // NCORES  # heads per core = 2
EL = HPC * HD  # local embed slice = 128
SB = S * B  # 4096 rows
SHARD = SB // NCORES  # 512 rows per core after reduce-scatter
QB = 1024  # qi block size

_CACHE: dict = {}
LAST_RESULT = None


def _build_program(with_cc: bool = True):
    import concourse.mybir as mybir
    import concourse.tile as tile
    from concourse import bacc
    from concourse.masks import make_identity

    f32 = mybir.dt.float32
    f32r = mybir.dt.float32r
    Exp = mybir.ActivationFunctionType.Exp
    Copy = mybir.ActivationFunctionType.Copy
    Ident = mybir.ActivationFunctionType.Identity
    add = mybir.AluOpType.add
    mult = mybir.AluOpType.mult

    nc = bacc.Bacc(
        "TRN2",
        target_bir_lowering=False,
        debug=False,
        enable_asserts=False,
        num_devices=NCORES,
    )

    def din(name, shape):
        return nc.dram_tensor(name, shape, f32, kind="ExternalInput").ap()

    query = din("query", [S, B, E])
    q_w = din("q_w", [E, EL])  # (Wq_slice * scaling).T
    k_w = din("k_w", [E, EL])
    v_w = din("v_w", [E, EL])
    o_w = din("o_w", [EL, E])  # Wo[:, slice].T
    bq_s = din("bq_s", [EL, 1])
    bk_s = din("bk_s", [EL, 1])
    bv_s = din("bv_s", [EL, 1])
    bo_in = din("bo_in", [1, E])  # bo/8 on every core (summed by the RS)
    cos_t = din("cos_t", [EL, S])  # 2-head stacked rope tables (sin sign-folded)
    sin_t = din("sin_t", [EL, S])
    out_ext = nc.dram_tensor("out", [SHARD, E], f32, kind="ExternalOutput").ap()

    with tile.TileContext(nc) as tc:
        with (
            tc.tile_pool(name="const", bufs=1) as const,
            tc.tile_pool(name="persist", bufs=1) as persist,
            tc.tile_pool(name="persistV", bufs=1) as persistV,
            tc.tile_pool(name="dram", bufs=1, space="DRAM") as dram,
        ):
            # ---- constants to SBUF (weights staged fp32, rounded to f32r) ----
            qw_sb = const.tile([128, 8, EL], f32r, name="qw_sb")
            kw_sb = const.tile([128, 8, EL], f32r, name="kw_sb")
            vw_sb = const.tile([128, 8, EL], f32r, name="vw_sb")
            ow_sb = const.tile([EL, E], f32r, name="ow_sb")
            bq_sb = const.tile([EL, 1], f32, name="bq_sb")
            bk_sb = const.tile([EL, 1], f32, name="bk_sb")
            bv_sb = const.tile([EL, 1], f32, name="bv_sb")
            bo_sb = const.tile([1, E], f32, name="bo_sb")
            bo_bc = const.tile([128, E], f32, name="bo_bc")
            ident = const.tile([128, 128], f32, name="ident")
            # two I_64 stacked on partitions 0:64 and 64:128 (for h=1 transposes)
            id64 = const.tile([128, HD], f32, name="id64")

            with tc.tile_pool(name="wstage", bufs=2) as wstage:
                for src, dst in ((q_w, qw_sb), (k_w, kw_sb), (v_w, vw_sb)):
                    stg = wstage.tile([128, 8, EL], f32, tag="wstg")
                    nc.sync.dma_start(stg[:], src.rearrange("(c p) m -> p c m", p=128))
                    nc.vector.tensor_copy(dst[:], stg[:])
                stg = wstage.tile([EL, E], f32, tag="owstg")
                nc.sync.dma_start(stg[:], o_w[:])
                nc.vector.tensor_copy(ow_sb[:], stg[:])

            nc.sync.dma_start(bq_sb[:], bq_s[:])
            nc.sync.dma_start(bk_sb[:], bk_s[:])
            nc.sync.dma_start(bv_sb[:], bv_s[:])
            nc.sync.dma_start(bo_sb[:], bo_in[:])
            make_identity(nc, ident[:])
            nc.vector.tensor_copy(id64[0:HD, :], ident[0:HD, 0:HD])
            nc.vector.tensor_copy(id64[HD:128, :], ident[0:HD, 0:HD])
            nc.gpsimd.partition_broadcast(bo_bc[:], bo_sb[:])

            # ---- persistent activations ----
            qT = persist.tile([EL, SB], f32r, name="qT")  # [2h*hd, b-major cols]
            kT = persist.tile([EL, SB], f32r, name="kT")
            vT = persist.tile([EL, SB], f32r, name="vT")
            # v kj-tiles [128, 64] + ones column, built during phase 1
            vaug = persistV.tile([128, HPC * B * 16, HD + 1], f32r, name="vaug")

            P_dram = [dram.tile([S, E], f32, name=f"P_dram{b}") for b in range(B)]
            rs_out = [
                dram.tile([S // NCORES, E], f32, name=f"rs_out{b}") for b in range(B)
            ]

            # ---- phase 1: transpose query, project, rope, v-tiles (fused) ----
            with (
                tc.tile_pool(name="ld", bufs=3) as ld,
                tc.tile_pool(name="qtb", bufs=2) as qtb,
                tc.tile_pool(name="tp_ps", bufs=3, space="PSUM") as tp_ps,
                tc.tile_pool(name="pj_ps", bufs=3, space="PSUM") as pj_ps,
                tc.tile_pool(name="vt_ps", bufs=1, space="PSUM") as vt_ps,
                tc.tile_pool(name="rope", bufs=2) as rope,
                tc.tile_pool(name="ropec", bufs=1) as ropec,
                tc.tile_pool(name="ones", bufs=1) as ones_pool,
            ):
                cos_sb = ropec.tile([EL, S], f32, name="cos_sb")
                sin_sb = ropec.tile([EL, S], f32, name="sin_sb")
                nc.sync.dma_start(cos_sb[:], cos_t[:])
                nc.sync.dma_start(sin_sb[:], sin_t[:])
                ones_f = ones_pool.tile([128, HPC * B * 16], f32, name="ones_f")
                nc.vector.memset(ones_f[:], 1.0)
                nc.vector.tensor_copy(vaug[:, :, HD], ones_f[:])

                for b in range(B):
                    for sblk in range(4):  # 512 s-rows per block
                        col0 = b * S + sblk * 512
                        qt_blk = qtb.tile([128, 8, 512], f32r, tag="qt_blk")
                        qry = ld.tile([128, 4, E], f32, tag="qry")
                        nc.sync.dma_start(
                            qry[:],
                            query[sblk * 512 : (sblk + 1) * 512, b].rearrange(
                                "(i p) e -> p i e", p=128
                            ),
                        )
                        for i in range(4):
                            # 4 transposes share one 1-bank psum tile, copied
                            # to SBUF in a single ACT op (ACT is idle here)
                            for eg in range(2):
                                tp = tp_ps.tile([128, 512], f32, tag="tp")
                                for ec2 in range(4):
                                    ec = eg * 4 + ec2
                                    nc.tensor.transpose(
                                        tp[:, ec2 * 128 : (ec2 + 1) * 128],
                                        qry[:, i, ec * 128 : (ec + 1) * 128],
                                        ident[:],
                                    )
                                nc.scalar.activation(
                                    qt_blk[
                                        :, eg * 4 : (eg + 1) * 4, i * 128 : (i + 1) * 128
                                    ],
                                    tp[:].rearrange("p (c m) -> p c m", c=4),
                                    Copy,
                                )
                        for w_sb, bias, dst, do_rope in (
                            (qw_sb, bq_sb, qT, True),
                            (kw_sb, bk_sb, kT, True),
                            (vw_sb, bv_sb, vT, False),
                        ):
                            ps = pj_ps.tile([128, 512], f32, tag="pj")
                            for ec in range(8):
                                nc.tensor.matmul(
                                    ps[:],
                                    w_sb[:, ec, :],
                                    qt_blk[:, ec, :],
                                    start=(ec == 0),
                                    stop=(ec == 7),
                                )
                            dcol = dst[:, col0 : col0 + 512]
                            nc.vector.tensor_scalar_add(dcol, ps[:], bias[:])
                            if do_rope:
                                # rope in-block: x' = x*cos + shuffle(x)*sin_f
                                ccol = slice(sblk * 512, (sblk + 1) * 512)
                                shuf = rope.tile([EL, 512], f32r, tag="shuf")
                                t1 = rope.tile([EL, 512], f32, tag="t1")
                                for h in range(HPC):
                                    p0 = h * HD
                                    nc.vector.tensor_copy(
                                        shuf[p0 : p0 + 32, :],
                                        dcol[p0 + 32 : p0 + 64, :],
                                    )
                                    nc.vector.tensor_copy(
                                        shuf[p0 + 32 : p0 + 64, :],
                                        dcol[p0 : p0 + 32, :],
                                    )
                                nc.vector.tensor_tensor(
                                    out=t1[:], in0=dcol, in1=cos_sb[:, ccol], op=mult
                                )
                                nc.vector.tensor_tensor(
                                    out=shuf[:], in0=shuf[:], in1=sin_sb[:, ccol], op=mult
                                )
                                nc.vector.tensor_tensor(
                                    out=dcol, in0=t1[:], in1=shuf[:], op=add
                                )
                            else:
                                # v natural kj-tiles for this block (both heads
                                # per psum tile; strided copy into vaug slots)
                                for kt2 in range(4):
                                    kt = sblk * 4 + kt2
                                    # separate psum tile per head: transposes
                                    # with different PE tile_positions must not
                                    # share a psum bank (hw fault otherwise)
                                    for h in range(HPC):
                                        vt = vt_ps.tile([128, HD], f32, tag=f"vt{h}")
                                        nc.tensor.transpose(
                                            vt[:],
                                            dcol[
                                                h * HD : (h + 1) * HD,
                                                kt2 * 128 : (kt2 + 1) * 128,
                                            ].bitcast(f32),
                                            id64[h * HD : (h + 1) * HD, :],
                                        )
                                        nc.scalar.activation(
                                            vaug[:, (h * B + b) * 16 + kt, :HD],
                                            vt[:],
                                            Copy,
                                        )

            # ---- attention-lifetime tiles (reuse freed SBUF) ----
            persist2_cm = tc.tile_pool(name="persist2", bufs=1)
            persist2 = persist2_cm.__enter__()
            attnT = [
                [
                    persist2.tile([EL, QB], f32r, name=f"attnT{b}_{qb}")
                    for qb in range(S // QB)
                ]
                for b in range(B)
            ]
            recip_bc = [
                [
                    persist2.tile([EL, QB], f32, name=f"recip_bc{b}_{qb}")
                    for qb in range(S // QB)
                ]
                for b in range(B)
            ]
            # all pairs' softmax row sums on partition 0, column-offset by pair
            sums_sb = persist2.tile([1, 4 * S], f32, name="sums_sb")

            # ---- phase 2: attention + normalize + out-projection, b-outer ----
            # PSUM budget: sc 2x2 + at 2x1 + op 1x2 = 8 banks.
            with (
                tc.tile_pool(name="sc_ps", bufs=2, space="PSUM") as sc_ps,
                tc.tile_pool(name="at_ps", bufs=1, space="PSUM") as at_ps,
                tc.tile_pool(name="op_ps", bufs=2, space="PSUM") as op_ps,
                tc.tile_pool(name="probs", bufs=4) as probs_pool,
                tc.tile_pool(name="osb", bufs=3) as osb,
            ):
                for b in range(B):
                    for qb in range(S // QB):
                        q0 = b * S + qb * QB
                        for h in range(HPC):
                            hs = slice(h * HD, (h + 1) * HD)
                            pair = h * B + b
                            at = at_ps.tile([HD + 1, QB], f32, tag="at")
                            for kt in range(16):
                                k0 = b * S + kt * 128
                                sc = sc_ps.tile([128, QB], f32, tag="sc")
                                for half in range(2):
                                    nc.tensor.matmul(
                                        sc[:, half * 512 : (half + 1) * 512],
                                        kT[hs, k0 : k0 + 128],
                                        qT[hs, q0 + half * 512 : q0 + (half + 1) * 512],
                                        start=True,
                                        stop=True,
                                        skip_group_check=True,
                                    )
                                pr = probs_pool.tile([128, QB], f32r, tag="pr")
                                nc.scalar.activation(pr[:], sc[:], Exp)
                                for half in range(2):
                                    nc.tensor.matmul(
                                        at[:, half * 512 : (half + 1) * 512],
                                        vaug[:, pair * 16 + kt, :],
                                        pr[:, half * 512 : (half + 1) * 512],
                                        start=(kt == 0),
                                        stop=(kt == 15),
                                        skip_group_check=True,
                                    )
                            nc.vector.tensor_copy(attnT[b][qb][hs, :], at[:HD, :])
                            nc.vector.tensor_copy(
                                sums_sb[
                                    0:1, pair * S + qb * QB : pair * S + (qb + 1) * QB
                                ],
                                at[HD : HD + 1, :],
                            )
                        # normalize this qi block, project it out.
                        # partition_broadcast honors neither in nor out
                        # partition bases -> broadcast to a base-0 stage and
                        # DVE-copy into the h=1 half.
                        for h in range(HPC):
                            pair = h * B + b
                            srow = sums_sb[
                                0:1, pair * S + qb * QB : pair * S + (qb + 1) * QB
                            ]
                            nc.vector.reciprocal(srow, srow)
                            if h == 0:
                                nc.gpsimd.partition_broadcast(
                                    recip_bc[b][qb][0:HD, :], srow
                                )
                            else:
                                rstage = osb.tile([HD, QB], f32, tag="rstage")
                                nc.gpsimd.partition_broadcast(rstage[:], srow)
                                nc.vector.tensor_copy(
                                    recip_bc[b][qb][HD : 2 * HD, :], rstage[:]
                                )
                        nc.vector.tensor_tensor(
                            out=attnT[b][qb][:],
                            in0=attnT[b][qb][:],
                            in1=recip_bc[b][qb][:],
                            op=mult,
                        )
                        for st2 in range(QB // 128):
                            st = qb * (QB // 128) + st2
                            for nch in range(2):
                                ps = op_ps.tile([128, 512], f32, tag="op")
                                nc.tensor.matmul(
                                    ps[:],
                                    attnT[b][qb][:, st2 * 128 : (st2 + 1) * 128],
                                    ow_sb[:, nch * 512 : (nch + 1) * 512],
                                    start=True,
                                    stop=True,
                                    skip_group_check=True,
                                )
                                psb = osb.tile([128, 512], f32, tag="ptile")
                                # fold bo/8 into this core's partial P
                                nc.vector.tensor_tensor(
                                    out=psb[:],
                                    in0=ps[:],
                                    in1=bo_bc[:, nch * 512 : (nch + 1) * 512],
                                    op=add,
                                )
                                nc.sync.dma_start(
                                    P_dram[b][
                                        st * 128 : (st + 1) * 128,
                                        nch * 512 : (nch + 1) * 512,
                                    ],
                                    psb[:],
                                )

            # ---- phase 3: per-batch reduce-scatter, shards straight out ----
            out_v = out_ext.rearrange("(s b) e -> s b e", b=B)
            for b in range(B):
                if with_cc:
                    nc.gpsimd.collective_compute(
                        "ReduceScatter",
                        add,
                        replica_groups=[list(range(NCORES))],
                        ins=[P_dram[b].opt()],
                        outs=[rs_out[b].opt()],
                    )
                else:  # timeline-sim variant: no collective, copy shard 0
                    nc.sync.dma_start(rs_out[b][:], P_dram[b][0 : S // NCORES, :])
                nc.sync.dma_start(out_v[:, b, :], rs_out[b][:])
            persist2_cm.__exit__(None, None, None)

    nc.compile()
    return nc


def _host_inputs(query, Wq, bq, Wk, bk, Wv, bv, Wo, bo):
    """Per-core input maps (all fp32, C-contiguous)."""
    scaling = HD ** (-0.5)

    invf = 1.0 / (
        10000.0 ** (np.arange(0, HD, 2, dtype=np.float32) / np.float32(HD))
    )
    t = np.arange(S, dtype=np.float32)
    fr = np.outer(t, invf).astype(np.float32)  # [S, 32]
    emb = np.concatenate([fr, fr], axis=1)  # [S, HD]
    cosT = np.cos(emb).T.astype(np.float32)  # [HD, S]
    sinT = np.sin(emb).T.astype(np.float32)
    sign = np.where(np.arange(HD) < HD // 2, -1.0, 1.0).astype(np.float32)[:, None]
    cos_t = np.ascontiguousarray(np.tile(cosT, (HPC, 1)), dtype=np.float32)
    sin_t = np.ascontiguousarray(np.tile(sinT * sign, (HPC, 1)), dtype=np.float32)

    query = np.ascontiguousarray(query, dtype=np.float32)
    bo8 = (np.asarray(bo, dtype=np.float32) / NCORES).reshape(1, E)
    in_maps = []
    for c in range(NCORES):
        sl = slice(c * EL, (c + 1) * EL)
        in_maps.append(
            {
                "query": query,
                "q_w": np.ascontiguousarray((Wq[sl, :] * scaling).T, dtype=np.float32),
                "k_w": np.ascontiguousarray(Wk[sl, :].T, dtype=np.float32),
                "v_w": np.ascontiguousarray(Wv[sl, :].T, dtype=np.float32),
                "o_w": np.ascontiguousarray(Wo[:, sl].T, dtype=np.float32),
                "bq_s": np.ascontiguousarray(
                    (bq[sl] * scaling).reshape(EL, 1), dtype=np.float32
                ),
                "bk_s": np.ascontiguousarray(bk[sl].reshape(EL, 1), dtype=np.float32),
                "bv_s": np.ascontiguousarray(bv[sl].reshape(EL, 1), dtype=np.float32),
                "bo_in": np.ascontiguousarray(bo8, dtype=np.float32),
                "cos_t": cos_t,
                "sin_t": sin_t,
            }
        )
    return in_maps


def kernel(query, Wq, bq, Wk, bk, Wv, bv, Wo, bo):
    global LAST_RESULT
    from concourse.bass_utils import run_bass_kernel_spmd

    if "nc" not in _CACHE:
        _CACHE["nc"] = _build_program()
    nc = _CACHE["nc"]

    in_maps = _host_inputs(
        np.asarray(query),
        np.asarray(Wq),
        np.asarray(bq),
        np.asarray(Wk),
        np.asarray(bk),
        np.asarray(Wv),
        np.asarray(bv),
        np.asarray(Wo),
        np.asarray(bo),
    )
    res = run_bass_kernel_spmd(nc, in_maps, core_ids=list(range(NCORES)))
    LAST_RESULT = res
    shards = [
        res.results[c]["out"].reshape(S // NCORES, B, E) for c in range(NCORES)
    ]
    return np.concatenate(shards, axis=0)

